# revision 12
# baseline (speedup 1.0000x reference)
"""BiAttention TRN2 kernel: data-parallel over batch across 8 NeuronCores.

Self-contained: hardcodes B=32, Tc=2048, Tq=256, D=256, 8 cores, 4 batches/core.
Raw-bass software-pipelined kernel; f32r matmuls; exact power-of-two mask trick.
"""
import numpy as np

import concourse.bass as bass
from concourse import mybir
from concourse.bass_utils import run_bass_kernel_spmd

F32 = mybir.dt.float32
F32R = mybir.dt.float32r
BF16 = mybir.dt.bfloat16
Exp = mybir.ActivationFunctionType.Exp
AX = mybir.AxisListType
OP = mybir.AluOpType

B, TC, TQ, D = 32, 2048, 256, 256
NCORES = 8
NB = B // NCORES          # batches per core = 4
NBLK = TC // 128          # c-blocks per batch = 16
NEG = -(2.0 ** 96)
SQ = 2.0 ** 48


def build_program():
    nc = bass.Bass()
    c_d = nc.declare_dram_parameter("c", [NB, TC, D], F32, isOutput=False)
    q_d = nc.declare_dram_parameter("q", [NB, TQ, D], F32, isOutput=False)
    mc_d = nc.declare_dram_parameter("mcf", [NB, 2, TC], F32, isOutput=False)
    mq_d = nc.declare_dram_parameter("mqf", [NB, 2, TQ], F32, isOutput=False)
    id_d = nc.declare_dram_parameter("ident", [128, 128], F32, isOutput=False)
    onew_d = nc.declare_dram_parameter("onesw", [128, 256], F32, isOutput=False)
    c100_d = nc.declare_dram_parameter("c100", [128, 1], F32, isOutput=False)

    o_d = nc.declare_dram_parameter("o", [NB, TC, D], F32, isOutput=True)
    qc_d = nc.declare_dram_parameter("qc", [NB, TQ], F32, isOutput=True)

    from contextlib import ExitStack
    es = ExitStack()
    _ctr = [0]

    def sb(shape, dt, name=None):
        _ctr[0] += 1
        return es.enter_context(nc.sbuf_tensor(name or f"sb{_ctr[0]}", shape, dt))

    def ps(shape, dt, name=None):
        _ctr[0] += 1
        return es.enter_context(nc.psum_tensor(name or f"ps{_ctr[0]}", shape, dt))

    def sem(name):
        return es.enter_context(nc.semaphore(name))

    # ---- SBUF ----
    cb = [sb([128, NBLK, D], F32R) for _ in range(2)]      # C natural (f32r), per-batch parity
    qn = [sb([128, 2, D], F32R) for _ in range(2)]          # Q natural [q%128, qchunk, d]
    qtr = [sb([128, 2, TQ], F32R) for _ in range(2)]        # Q^T [d%128, dchunk, q]
    mcs = [sb([2, TC], F32R) for _ in range(2)]             # mask lhsT features
    mqs = [sb([2, TQ], F32R) for _ in range(2)]             # mask rhs features
    ident = sb([128, 128], F32R)
    ones_w = sb([128, 256], F32R)                           # all-ones (total-sum rhs)
    c100 = sb([128, 1], F32)                                # bias constant -100
    ctr = [sb([128, 2, 128], F32R) for _ in range(2)]       # C^T chunks, block parity
    ptr = [sb([128, 2, 128], BF16) for _ in range(2)]       # P^T chunks (bf16), block parity
    p_sb = [sb([128, TQ], BF16) for _ in range(2)]          # exp(S-m) (bf16), block parity
    qn_b = [sb([128, 2, D], BF16) for _ in range(2)]        # Q natural bf16 (mm2 rhs)
    ident_b = sb([128, 128], BF16)
    o_all = [sb([128, NBLK, D], F32) for _ in range(2)]     # output batch buffer
    NM = [sb([128, NBLK], F32) for _ in range(2)]           # -rowmax per block column
    SS = [sb([128, NBLK], F32) for _ in range(2)]           # rowsum per block column
    RS = [sb([128, NBLK], F32) for _ in range(2)]           # 1/rowsum
    E_all = sb([128, NBLK], F32R)                           # exp(m - 100) for q2c
    esum = sb([128, 1], F32)
    esum_r = sb([128, 1], F32R)
    t_sb = sb([1, 1], F32)
    rtot = sb([1, 1], F32)
    qc_sb = [sb([1, TQ], F32) for _ in range(2)]

    # ---- PSUM (bank-granular allocator: 8 banks total) ----
    pJ = [ps([128, 512], F32R) for _ in range(2)]   # CT at [:,0:256], PT at [:,256:512]
    pS = [ps([128, 256], F32) for _ in range(2)]
    pO = [ps([128, 256], F32) for _ in range(2)]
    pQT = ps([128, 2, 256], F32R)                   # batch prep QT; tail nmin-transpose at [0:1,0,0:128]
    # pM regions: pQC=[0:1,0:256], pTot=[0:1,256:512]
    pM = ps([128, 512], F32)

    sems = {}
    for name in ("s_cin", "s_out", "s_qc", "pe_ct", "pe_qt", "pe_s", "pe_pt",
                 "pe_o", "pt_", "dve_ctr", "dve_qtr", "dve_nm", "dve_rs",
                 "dve_ptr", "dt", "act_p", "act_o", "at", "s_misc"):
        sems[name] = sem(name)
    s_cin = sems["s_cin"]; s_out = sems["s_out"]; s_qc = sems["s_qc"]
    pe_ct = sems["pe_ct"]; pe_qt = sems["pe_qt"]; pe_s = sems["pe_s"]
    pe_pt = sems["pe_pt"]; pe_o = sems["pe_o"]; pt_ = sems["pt_"]
    dve_ctr = sems["dve_ctr"]; dve_qtr = sems["dve_qtr"]; dve_nm = sems["dve_nm"]
    dve_rs = sems["dve_rs"]; dve_ptr = sems["dve_ptr"]; dt = sems["dt"]
    act_p = sems["act_p"]; act_o = sems["act_o"]; at = sems["at"]
    s_misc = sems["s_misc"]

    blk = es.enter_context(nc.Block())
    with blk:
        # ---------------- GPSIMD: input cast-DMAs ----------------
        @blk.gpsimd
        def _(g):
            for b in range(NB):
                if b >= 2:
                    g.wait_ge(pt_, b - 1)
                if b >= 1:
                    # all previously issued input DMAs must have completed so
                    # cumulative thresholds are meaningful (unordered DMA completion)
                    g.wait_ge(s_cin, 64 * b + 48)
                g.dma_start(cb[b % 2][:], c_d[b].rearrange("(i p) d -> p i d", p=128)).then_inc(s_cin, 16)
                g.dma_start(qn[b % 2][:], q_d[b].rearrange("(a p) d -> p a d", p=128)).then_inc(s_cin, 16)
                g.dma_start(mcs[b % 2][:], mc_d[b]).then_inc(s_cin, 16)
                g.dma_start(mqs[b % 2][:], mq_d[b]).then_inc(s_cin, 16)
                if b == 0:
                    g.dma_start(ident[:], id_d[:]).then_inc(s_cin, 16)
                    g.dma_start(ones_w[:], onew_d[:]).then_inc(s_cin, 16)
                    g.dma_start(c100[:], c100_d[:]).then_inc(s_cin, 16)

        def cin_thresh(b):
            return 64 * (b + 1) + 48

        # ---------------- PE ----------------
        @blk.tensor
        def _(t):
            def ct_tr(n):
                b, i = divmod(n, NBLK)
                k = n % 2
                if i == 0:
                    t.wait_ge(s_cin, cin_thresh(b))
                tr0 = t.transpose(pJ[k][:, 0:128], cb[b % 2][:, i, 0:128], ident[:])
                if n >= 1:
                    tr0._wait_ge(dve_ptr, n - 1)
                t.transpose(pJ[k][:, 128:256], cb[b % 2][:, i, 128:256], ident[:]).then_inc(pe_ct, 1)

            def sim(n):
                b, i = divmod(n, NBLK)
                k = n % 2
                t.wait_ge(dve_ctr, n + 1)
                if i == 0:
                    t.wait_ge(dve_qtr, b + 1)
                mm0 = t.matmul(pS[k][:], mcs[b % 2][:, i * 128:(i + 1) * 128], mqs[b % 2][:],
                               start=True, stop=False)
                if n >= 2:
                    mm0._wait_ge(act_p, n - 1)
                t.matmul(pS[k][:], ctr[k][:, 0], qtr[b % 2][:, 0], start=False, stop=False)
                t.matmul(pS[k][:], ctr[k][:, 1], qtr[b % 2][:, 1], start=False, stop=True).then_inc(pe_s, 1)

            def pt_tr(n):
                k = n % 2
                if n == 0:
                    t.wait_ge(s_misc, 1)    # ident_b ready
                if n >= 2:
                    t.wait_ge(dve_ptr, n - 1)
                ptb = pJ[k][:].bitcast(BF16)
                tr0 = t.transpose(ptb[:, 512:640], p_sb[k][:, 0:128], ident_b[:])
                tr0._wait_ge(act_p, n + 1)
                t.transpose(ptb[:, 640:768], p_sb[k][:, 128:256], ident_b[:]).then_inc(pe_pt, 1)

            def mm2(n):
                b, i = divmod(n, NBLK)
                k = n % 2
                if n >= 2:
                    t.wait_ge(act_o, n - 1)
                mm0 = t.matmul(pO[k][:], ptr[k][:, 0], qn_b[b % 2][:, 0], start=True, stop=False)
                mm0._wait_ge(dve_ptr, n + 1)
                t.matmul(pO[k][:], ptr[k][:, 1], qn_b[b % 2][:, 1], start=False, stop=True).then_inc(pe_o, 1)

            def qt_prep(b):
                t.wait_ge(s_cin, cin_thresh(b))
                if b >= 1:
                    t.wait_ge(dve_qtr, b)   # prev batch qtr copy done (pQT bank free)
                last = None
                for qa in range(2):
                    for kk in range(2):
                        last = t.transpose(
                            pQT[:, kk, qa * 128:(qa + 1) * 128],
                            qn[b % 2][:, qa, kk * 128:(kk + 1) * 128],
                            ident[:],
                        )
                last.then_inc(pe_qt, 1)

            def tail(b):
                # C: q2c matmuls + total sum (constant-shift exp, no global max)
                t.wait_ge(dt, 2 * b + 1)      # esum_r ready
                t.wait_ge(at, 2 * b + 1)      # E_all ready
                if b >= 1:
                    t.wait_ge(at, 2 * b)      # T2(b-1) done reading pM
                for i in range(NBLK):
                    t.matmul(pM[0:1, 0:256], E_all[:, i:i + 1], cb[b % 2][:, i, :],
                             start=(i == 0), stop=(i == NBLK - 1))
                t.matmul(pM[0:1, 256:512], esum_r[:], ones_w[:], start=True,
                         stop=True).then_inc(pt_, 1)

            for b in range(NB):
                qt_prep(b)
                for slot in range(NBLK + 3):
                    i = slot - 2
                    if 0 <= i <= NBLK - 1:
                        pt_tr(16 * b + i)
                    i = slot - 3
                    if 0 <= i <= NBLK - 1:
                        mm2(16 * b + i)
                    i = slot
                    if 0 <= i <= NBLK - 1:
                        ct_tr(16 * b + i)
                    i = slot - 1
                    if 0 <= i <= NBLK - 1:
                        sim(16 * b + i)
                tail(b)

        # ---------------- DVE ----------------
        @blk.vector
        def _(v):
            def qtr_copy(b):
                if b == 0:
                    v.wait_ge(s_cin, cin_thresh(0))
                    v.tensor_copy(ident_b[:], ident[:]).then_inc(s_misc, 1)
                v.wait_ge(pe_qt, b + 1)
                if b >= 2:
                    v.wait_ge(pe_o, 16 * (b - 1))   # qn_b WAR (implies pe_s too)
                v.tensor_copy(qn_b[b % 2][:], qn[b % 2][:])
                v.tensor_copy(qtr[b % 2][:], pQT[:]).then_inc(dve_qtr, 1)

            def ctr_copy(n):
                k = n % 2
                if n >= 2:
                    v.wait_ge(pe_s, n - 1)
                cp = v.tensor_copy(ctr[k][:], pJ[k][:, 0:256])
                cp._wait_ge(pe_ct, n + 1)
                cp.then_inc(dve_ctr, 1)

            def nm(n):
                b, i = divmod(n, NBLK)
                k = n % 2
                if i == 0 and b >= 2:
                    v.wait_ge(at, 2 * (b - 2) + 1)   # tail(b-2) E-exp read NM buffer
                rd = v.tensor_reduce(NM[b % 2][:, i:i + 1], pS[k][:], AX.X, OP.max,
                                     negate=True)
                rd._wait_ge(pe_s, n + 1)
                rd.then_inc(dve_nm, 1)

            def ptr_copy(n):
                k = n % 2
                if n >= 2:
                    v.wait_ge(pe_o, n - 1)
                cp = v.tensor_copy(ptr[k][:], pJ[k][:].bitcast(BF16)[:, 512:768])
                cp._wait_ge(pe_pt, n + 1)
                cp.then_inc(dve_ptr, 1)

            def recip(n):
                b, i = divmod(n, NBLK)
                if i == 0 and b >= 2:
                    v.wait_ge(act_o, 16 * (b - 1))   # RS WAR vs out-copy of b-2
                rc = v.reciprocal(RS[b % 2][:, i:i + 1], SS[b % 2][:, i:i + 1])
                rc._wait_ge(act_p, n + 1)
                rc.then_inc(dve_rs, 1)

            def tail(b):
                # X1: esum -> f32r
                v.wait_ge(at, 2 * b + 1)
                v.tensor_copy(esum_r[:], esum[:]).then_inc(dt, 1)
                # X2: total -> reciprocal
                v.wait_ge(pt_, b + 1)
                if b >= 1:
                    v.wait_ge(at, 2 * b)   # T2(b-1) done with rtot
                v.tensor_copy(t_sb[:], pM[0:1, 256:257])
                v.drain()
                v.reciprocal(rtot[:], t_sb[:]).then_inc(dt, 1)

            for b in range(NB):
                qtr_copy(b)
                for slot in range(NBLK + 3):
                    i = slot - 2
                    if 0 <= i <= NBLK - 1:
                        ptr_copy(16 * b + i)
                        recip(16 * b + i)
                    i = slot
                    if 0 <= i <= NBLK - 1:
                        ctr_copy(16 * b + i)
                    i = slot - 1
                    if 0 <= i <= NBLK - 1:
                        nm(16 * b + i)
                tail(b)

        # ---------------- ACT ----------------
        @blk.scalar
        def _(s):
            def ex(n):
                b, i = divmod(n, NBLK)
                k = n % 2
                if n >= 2:
                    s.wait_ge(pe_pt, n - 1)
                if i == 0 and b >= 2:
                    s.wait_ge(dve_rs, 16 * (b - 1))  # SS WAR vs recip of b-2
                ac = s.activation(p_sb[k][:], pS[k][:], Exp,
                                  bias=NM[b % 2][:, i:i + 1],
                                  accum_out=SS[b % 2][:, i:i + 1])
                ac._wait_ge(dve_nm, n + 1)
                ac.then_inc(act_p, 1)

            def outcp(n):
                b, i = divmod(n, NBLK)
                k = n % 2
                s.wait_ge(dve_rs, n + 1)
                if i == 0 and b >= 2:
                    s.wait_ge(s_out, 16 * (b - 1))
                oc = s.mul(o_all[b % 2][:, i, :], pO[k][:], RS[b % 2][:, i:i + 1])
                oc._wait_ge(pe_o, n + 1)
                oc.then_inc(act_o, 1)

            def tail(b):
                # T1: E = exp(-NM - 100), accum esum
                s.wait_ge(dve_nm, 16 * (b + 1))
                if b >= 1:
                    s.wait_ge(pt_, b)        # E_all/esum WAR vs tail C of b-1
                s.activation(E_all[:], NM[b % 2][:], Exp, bias=c100[:], scale=-1.0,
                             accum_out=esum[:]).then_inc(at, 1)
                # T2: qc = pQC * rtot
                s.wait_ge(dt, 2 * b + 2)
                s.wait_ge(pt_, b + 1)
                if b >= 2:
                    s.wait_ge(s_qc, 16 * (b - 1))
                s.mul(qc_sb[b % 2][:], pM[0:1, 0:256], rtot[:]).then_inc(at, 1)

            for b in range(NB):
                for slot in range(NBLK + 3):
                    i = slot - 1
                    if 0 <= i <= NBLK - 1:
                        ex(16 * b + i)
                    i = slot - 3
                    if 0 <= i <= NBLK - 1:
                        outcp(16 * b + i)
                tail(b)

        # ---------------- SYNC: output DMAs ----------------
        @blk.sync
        def _(sy):
            for b in range(NB):
                sy.wait_ge(act_o, 16 * (b + 1))
                if b >= 1:
                    sy.wait_ge(s_out, 16 * b)
                sy.dma_start(o_d[b].rearrange("(i p) d -> p i d", p=128),
                             o_all[b % 2][:]).then_inc(s_out, 16)
                sy.wait_ge(at, 2 * b + 2)
                if b >= 1:
                    sy.wait_ge(s_qc, 16 * b)
                sy.dma_start(qc_d[b:b + 1, :], qc_sb[b % 2][:]).then_inc(s_qc, 16)

    return nc, es


_CACHE = {}


def _get_program():
    if "nc" not in _CACHE:
        nc, es = build_program()
        _CACHE["nc"] = nc
        _CACHE["es"] = es
    return _CACHE["nc"]


def kernel(context_repr, question_repr, context_len, question_len):
    context_repr = np.ascontiguousarray(np.asarray(context_repr, np.float32))
    question_repr = np.ascontiguousarray(np.asarray(question_repr, np.float32))
    context_len = np.asarray(context_len, np.int32)
    question_len = np.asarray(question_len, np.int32)

    cm = (np.arange(TC)[None, :] < context_len[:, None]).astype(np.float32)  # [B,Tc]
    qm = (np.arange(TQ)[None, :] < question_len[:, None]).astype(np.float32)  # [B,Tq]
    mcf = np.stack([SQ * cm, np.ones_like(cm)], axis=1)                      # [B,2,Tc]
    mqf = np.stack([SQ * qm, np.full_like(qm, NEG)], axis=1)                 # [B,2,Tq]
    ident = np.eye(128, dtype=np.float32)
    onesw = np.ones((128, 256), np.float32)
    c100 = np.full((128, 1), -100.0, np.float32)

    nc = _get_program()
    in_maps = []
    for core in range(NCORES):
        sl = slice(core * NB, (core + 1) * NB)
        in_maps.append({
            "c": context_repr[sl],
            "q": question_repr[sl],
            "mcf": np.ascontiguousarray(mcf[sl]),
            "mqf": np.ascontiguousarray(mqf[sl]),
            "ident": ident,
            "onesw": onesw,
            "c100": c100,
        })

    res = run_bass_kernel_spmd(nc, in_maps, list(range(NCORES)))
    out1 = np.concatenate([np.asarray(r["o"]).reshape(NB, TC, D) for r in res.results], axis=0)
    q2c = np.concatenate([np.asarray(r["qc"]).reshape(NB, TQ) for r in res.results], axis=0)
    out2 = np.broadcast_to(q2c[:, None, :], (B, TC, D))
    return out1, out2
